# revision 1
# baseline (speedup 1.0000x reference)
"""DiagonalQuadratic forward: y = sum(Q * x * x, -1) + x @ b + c for x [131072, 512].

Strategy (8-core data parallel, 16384 rows/core):
  y_n = sum_d Q_d x_nd^2 + b_d x_nd + c
      = sum_d sign_d * (s_d x_nd + t_d)^2 + K        (complete the square)
  with s_d = sqrt(|Q_d|), t_d = sign_d b_d / (2 s_d), K = c - sum_d sign_d t_d^2.

Per core, per block of 1024 rows:
  - DMA x block to SBUF [128 part, 8 rows * 512] - each partition holds 8
    consecutive rows = one contiguous 16KB DRAM read per partition
  - PE transpose 128x128 chunks so d lands on partitions (fp32, bit-exact)
  - ACT: z = Square(s_d * x_t + t_d) with per-partition scale/bias -> f32r
  - PE matmul (f32r, 1 cyc/row): y[1, n] += sign[128,1].T @ z[128, n]
  - DVE adds K, GPSIMD DMA out. Host undoes the row permutation on reshape.

Columns where |Q| is tiny (completion ill-conditioned) are zeroed on-device
and corrected exactly on the host (empty set for the reference distribution).
"""

import sys

if "/opt/trn_rl_repo" not in sys.path:
    sys.path.insert(0, "/opt/trn_rl_repo")

import numpy as np
from contextlib import ExitStack

import concourse.bacc as bacc
import concourse.tile as tile
import concourse.mybir as mybir
from concourse import masks
from concourse.bass_utils import run_bass_kernel_spmd

F16 = mybir.dt.float16
F32 = mybir.dt.float32
F32R = mybir.dt.float32r

N_TOTAL = 131072
D = 512
N_CORES = 8
N_PC = N_TOTAL // N_CORES       # 16384 rows per core
BLK_N = 1024                    # rows per block
N_BLK = N_PC // BLK_N           # 16 blocks
R_PP = BLK_N // 128             # consecutive rows per partition per block
KCH = D // 128                  # 4 d-chunks
G = BLK_N // 512                # 2 matmul column groups per block

_CACHED_NC = None
_last_prm = None
_last_kc = None


def _build_nc():
    nc = bacc.Bacc("TRN2", target_bir_lowering=False, debug=False, num_devices=N_CORES)
    x_d = nc.dram_tensor("x", [N_PC, D], F32R, kind="ExternalInput")
    # packed params: cols 0:4 = s (sqrt|Q|) per d-chunk, 4:8 = t (bias), 8:12 = sign
    prm = nc.dram_tensor("prm", [128, 12], F32, kind="ExternalInput")
    kc = nc.dram_tensor("kc", [1, 1], F32, kind="ExternalInput")
    y_d = nc.dram_tensor("y", [N_BLK, BLK_N], F32, kind="ExternalOutput")

    # each partition holds R_PP consecutive rows -> one contiguous DRAM read
    # per partition per block
    x_blocks = x_d.ap().rearrange("(a p r) d -> a p r d", p=128, r=R_PP)

    with tile.TileContext(nc) as tc, ExitStack() as ctx:
        cpool = ctx.enter_context(tc.tile_pool(name="cpool", bufs=1))
        xpool = ctx.enter_context(tc.tile_pool(name="xpool", bufs=8))
        zpool = ctx.enter_context(tc.tile_pool(name="zpool", bufs=6))
        opool = ctx.enter_context(tc.tile_pool(name="opool", bufs=3))
        tps = ctx.enter_context(tc.tile_pool(name="tps", bufs=4, space="PSUM"))
        yps = ctx.enter_context(tc.tile_pool(name="yps", bufs=2, space="PSUM"))

        ident_f = cpool.tile([128, 128], F32)
        masks.make_identity(nc, ident_f[:])
        ident = cpool.tile([128, 128], F32R)
        nc.scalar.copy(ident[:], ident_f[:])
        prm_sb = cpool.tile([128, 12], F32)
        nc.gpsimd.dma_start(prm_sb[:], prm[:])
        kc_sb = cpool.tile([1, 1], F32)
        nc.gpsimd.dma_start(kc_sb[:], kc[:])
        sgn_r = cpool.tile([128, 4], F32R)
        nc.scalar.copy(sgn_r[:], prm_sb[:, 8:12])

        for blk in range(N_BLK):
            x_sb = xpool.tile([128, R_PP * D], F32R)
            half = R_PP // 2
            for hh in range(2):
                nc.sync.dma_start(
                    x_sb[:, hh * half * D : (hh + 1) * half * D].rearrange(
                        "p (r d) -> p r d", d=D),
                    x_blocks[blk][:, hh * half : (hh + 1) * half],
                )

            y_ps = yps.tile([1, BLK_N], F32)
            for k in range(KCH):
                for g in range(G):
                    t_ps = tps.tile([128, 512], F32R, tag="t_ps")
                    for rr in range(4):
                        r = 4 * g + rr
                        nc.tensor.transpose(
                            t_ps[:, 128 * rr : 128 * (rr + 1)],
                            x_sb[:, r * D + 128 * k : r * D + 128 * (k + 1)],
                            ident[:],
                        )
                    z = zpool.tile([128, 512], F32R, tag="z")
                    nc.scalar.activation(
                        z[:], t_ps[:], mybir.ActivationFunctionType.Square,
                        bias=prm_sb[:, 4 + k : 5 + k], scale=prm_sb[:, k : k + 1],
                    )
                    nc.tensor.matmul(
                        y_ps[0:1, 512 * g : 512 * (g + 1)],
                        sgn_r[:, k : k + 1], z[:],
                        start=(k == 0), stop=(k == KCH - 1),
                    )
            y_sb = opool.tile([1, BLK_N], F32)
            nc.vector.tensor_scalar_add(y_sb[:], y_ps[:], kc_sb[0:1, 0:1])
            nc.gpsimd.dma_start(y_d[blk : blk + 1, :], y_sb[:])

    nc.compile()
    return nc


def kernel(x, Q, b, c):
    global _CACHED_NC
    x32 = np.ascontiguousarray(np.asarray(x, dtype=np.float32))
    Q64 = np.asarray(Q, dtype=np.float64)
    b64 = np.asarray(b, dtype=np.float64)
    c64 = float(np.asarray(c, dtype=np.float64).reshape(-1)[0])

    absQ = np.abs(Q64)
    # ill-conditioned columns: completion amplifies b^2/(4|Q|); keep device-side
    # values bounded and fix up exactly on host.
    with np.errstate(divide="ignore", invalid="ignore"):
        amp = np.where(absQ > 0, b64 * b64 / (4 * absQ), np.inf)
    bad = (amp > 2000.0) | (absQ == 0.0)

    sgn = np.where(np.asarray(Q) >= 0, 1.0, -1.0).astype(np.float32)
    s = np.sqrt(absQ).astype(np.float32)
    with np.errstate(divide="ignore", invalid="ignore"):
        t = (sgn.astype(np.float64) * b64 / (2 * s.astype(np.float64))).astype(np.float32)
    sgn[bad] = 0.0
    s[bad] = 0.0
    t[bad] = 0.0
    K = np.float32(c64 - np.sum(sgn.astype(np.float64) * t.astype(np.float64) ** 2))

    prm = np.zeros((128, 12), dtype=np.float32)
    prm[:, 0:4] = s.reshape(4, 128).T
    prm[:, 4:8] = t.reshape(4, 128).T
    prm[:, 8:12] = sgn.reshape(4, 128).T
    kc = np.full((1, 1), K, dtype=np.float32)

    global _last_prm, _last_kc
    _last_prm, _last_kc = prm, kc

    if _CACHED_NC is None:
        _CACHED_NC = _build_nc()
    nc = _CACHED_NC

    in_maps = [
        {"x": x32[i * N_PC : (i + 1) * N_PC], "prm": prm, "kc": kc}
        for i in range(N_CORES)
    ]
    out = run_bass_kernel_spmd(nc, in_maps, core_ids=list(range(N_CORES)))
    parts = []
    for r in out.results:
        # y_dev[blk, 512*g + 128*rr + p] = y[n0 + R_PP*p + 4*g + rr]
        yb = r["y"].reshape(N_BLK, G, 4, 128)
        parts.append(yb.transpose(0, 3, 1, 2).reshape(-1))
    y = np.concatenate(parts)

    if bad.any():
        idx = np.nonzero(bad)[0]
        xs = x32[:, idx].astype(np.float64)
        corr = (xs * xs) @ Q64[idx] + xs @ b64[idx]
        y = y + corr.astype(np.float32)

    return y.reshape(N_TOTAL, 1).astype(np.float32)



# revision 20
# speedup vs baseline: 1.0633x; 1.0633x over previous
"""DiagonalQuadratic forward: y = sum(Q * x * x, -1) + x @ b + c for x [131072, 512].

Strategy (8-core data parallel, 16384 rows/core, fp16 device path):
  y_n = sum_d sign_d * (s_d x_nd + t_d)^2 + K        (complete the square)
  with s_d = sqrt(|Q_d|), t_d = sign_d b_d / (2 s_d), K = c - sum_d sign_d t_d^2.

x is cast to fp16 on the host (staging choice; rel err ~1e-4 << 2e-2 gate).
Per core, per block of 1024 rows:
  - d-chunk 0 (cols 0:128) arrives TRANSPOSED via the DMA xbar
    (dma_start_transpose, 16x128 tiles) directly into SBUF [128, 1024].
  - d-chunks 1..3 (cols 128:512) arrive raw: partition p holds rows
    {128r + p} (r outer) so PE-transposed tiles come out in natural row
    order; PE transposes them into fp16 PSUM.
  - square: chunk 0,1 on DVE (tensor_scalar mult+add then tensor_tensor
    self-mult, 2x/4x modes), chunks 2,3 on ACT (fused Square(s*x+t)).
  - PE matmuls sgn[128,1].T @ z[128,512] accumulate y into PSUM
    partitions 0 (cols 0:512) and 64 (cols 512:1024).
  - GPSIMD copies y PSUM->SBUF and DMAs out. Host adds K.

Columns where |Q| is tiny (completion ill-conditioned) are zeroed on-device
and corrected exactly on the host (empty set for the reference distribution).
"""

import sys

if "/opt/trn_rl_repo" not in sys.path:
    sys.path.insert(0, "/opt/trn_rl_repo")

import numpy as np
from contextlib import ExitStack

import concourse.bacc as bacc
import concourse.tile as tile
import concourse.mybir as mybir
from concourse import masks
from concourse.bass_utils import run_bass_kernel_spmd

F16 = mybir.dt.float16
F32 = mybir.dt.float32

N_TOTAL = 131072
D = 512
N_CORES = 8
N_PC = N_TOTAL // N_CORES       # 16384 rows per core
BLK_N = 1024                    # rows per block
N_BLK = N_PC // BLK_N           # 16 blocks
R_PP = BLK_N // 128             # 8 row-segments per partition per block
KCH = D // 128                  # 4 d-chunks; chunk 0 via xbar, 1..3 via PE

_CACHED_NC = None
USE_XBAR = True
DEBUG_TAPS = False
Y_COPY_DVE = True


def _build_nc():
    nc = bacc.Bacc("TRN2", target_bir_lowering=False, debug=False, num_devices=N_CORES)
    x_d = nc.dram_tensor("x", [N_PC, D], F16, kind="ExternalInput")
    # packed params: cols 0:4 = s (sqrt|Q|) per d-chunk, 4:8 = t (bias), 8:12 = sign
    prm = nc.dram_tensor("prm", [128, 12], F32, kind="ExternalInput")
    y_d = nc.dram_tensor("y", [N_BLK, BLK_N], F32, kind="ExternalOutput")
    if DEBUG_TAPS:
        zt_d = nc.dram_tensor("ztap", [2, KCH, 128, BLK_N], F16, kind="ExternalOutput")
        xt_d = nc.dram_tensor("xtap", [2, 128, R_PP * 384], F16, kind="ExternalOutput")
        yt_d = nc.dram_tensor("ytap", [2, 3, 512], F32, kind="ExternalOutput")

    x_ap = x_d.ap()
    # raw path: partition p holds rows {128r + p} so transposed tiles are in
    # natural row order; per partition 8 reads of 768B (cols 128:512)
    x_rows = x_ap.rearrange("(a r p) d -> a p r d", p=128, r=R_PP)

    with tile.TileContext(nc) as tc, ExitStack() as ctx:
        cpool = ctx.enter_context(tc.tile_pool(name="cpool", bufs=1))
        xpool = ctx.enter_context(tc.tile_pool(name="xpool", bufs=4))
        xbpool = ctx.enter_context(tc.tile_pool(name="xbpool", bufs=4))
        upool = ctx.enter_context(tc.tile_pool(name="upool", bufs=4))
        zpool = ctx.enter_context(tc.tile_pool(name="zpool", bufs=8))
        opool = ctx.enter_context(tc.tile_pool(name="opool", bufs=3))
        tps = ctx.enter_context(tc.tile_pool(name="tps", bufs=6, space="PSUM"))
        yps = ctx.enter_context(tc.tile_pool(name="yps", bufs=2, space="PSUM"))

        ident_f = cpool.tile([128, 128], F32)
        masks.make_identity(nc, ident_f[:])
        ident = cpool.tile([128, 128], F16)
        nc.scalar.copy(ident[:], ident_f[:])
        prm_sb = cpool.tile([128, 12], F32)
        nc.gpsimd.dma_start(prm_sb[:], prm[:])
        sgn16 = cpool.tile([128, 4], F16)
        nc.scalar.copy(sgn16[:], prm_sb[:, 8:12])

        # software pipeline: issue loads/transposes/squares for block b, then
        # reduce matmuls + store for block b-1, so PE never idles on z.
        state = {}

        def stage_front(blk):
            # loads
            if USE_XBAR:
                xt0 = xbpool.tile([128, BLK_N], F16, tag="xt0")
                nc.sync.dma_start_transpose(
                    xt0[:], x_ap[blk * BLK_N : (blk + 1) * BLK_N, 0:128]
                )
                ncol, c0 = 384, 128
            else:
                ncol, c0 = 512, 0
            x_sb = xpool.tile([128, R_PP * ncol], F16, tag="x_sb")
            nc.sync.dma_start(
                x_sb[:].rearrange("p (r c) -> p r c", c=ncol),
                x_rows[blk][:, :, c0:512],
            )
            # PE transposes -> fp16 PSUM, natural column order
            t_chunks = {}
            for k in range(0 if not USE_XBAR else 1, KCH):
                t_ps = tps.tile([128, BLK_N], F16, tag="t_ps")
                for rr in range(R_PP):
                    nc.tensor.transpose(
                        t_ps[:, 128 * rr : 128 * (rr + 1)],
                        x_sb[:, rr * ncol + (k * 128 - c0) : rr * ncol + (k * 128 - c0) + 128],
                        ident[:],
                    )
                t_chunks[k] = t_ps

            # squares -> z_k [128, 1024] f16 SBUF
            zs = []
            # chunk 0: DVE tensor_scalar (4x from SBUF / 2x from PSUM) + 2x tt
            src0 = xt0[:] if USE_XBAR else t_chunks[0][:]
            u0 = upool.tile([128, BLK_N], F16, tag="u0")
            nc.vector.tensor_scalar(
                u0[:], src0, prm_sb[:, 0:1], prm_sb[:, 4:5],
                mybir.AluOpType.mult, mybir.AluOpType.add,
            )
            z0 = zpool.tile([128, BLK_N], F16, tag="z0")
            nc.vector.tensor_tensor(z0[:], u0[:], u0[:], mybir.AluOpType.mult)
            zs.append(z0)
            # chunk 1 (PSUM source): DVE
            u1 = upool.tile([128, BLK_N], F16, tag="u1")
            nc.vector.tensor_scalar(
                u1[:], t_chunks[1][:], prm_sb[:, 1:2], prm_sb[:, 5:6],
                mybir.AluOpType.mult, mybir.AluOpType.add,
            )
            z1 = zpool.tile([128, BLK_N], F16, tag="z1")
            nc.vector.tensor_tensor(z1[:], u1[:], u1[:], mybir.AluOpType.mult)
            zs.append(z1)
            # chunks 2, 3 (PSUM source): ACT fused square
            for k in (2, 3):
                z = zpool.tile([128, BLK_N], F16, tag=f"z{k}")
                nc.scalar.activation(
                    z[:], t_chunks[k][:], mybir.ActivationFunctionType.Square,
                    bias=prm_sb[:, 4 + k : 5 + k], scale=prm_sb[:, k : k + 1],
                )
                zs.append(z)

            if DEBUG_TAPS and blk in (0, 3):
                ti = 0 if blk == 0 else 1
                nc.sync.dma_start(xt_d[ti], x_sb[:])
                for k in range(KCH):
                    nc.sync.dma_start(zt_d[ti, k], zs[k][:])

            state[blk] = zs

        def stage_back(blk):
            zs = state.pop(blk)
            # reduce: one [1, 512] PSUM bank per column group
            y_g = []
            for g in range(2):
                y_ps = yps.tile([1, 512], F32)
                for k in range(KCH):
                    nc.tensor.matmul(
                        y_ps[0:1, :],
                        sgn16[:, k : k + 1],
                        zs[k][:, 512 * g : 512 * (g + 1)],
                        start=(k == 0), stop=(k == KCH - 1),
                    )
                y_g.append(y_ps)
            y_sb = opool.tile([1, BLK_N], F32)
            nc.scalar.copy(y_sb[0:1, 0:512], y_g[0][0:1, :])
            if Y_COPY_DVE:
                nc.vector.tensor_copy(y_sb[0:1, 512:1024], y_g[1][0:1, :])
            else:
                nc.scalar.copy(y_sb[0:1, 512:1024], y_g[1][0:1, :])
            nc.gpsimd.dma_start(y_d[blk : blk + 1, :], y_sb[:])
            if DEBUG_TAPS and blk in (0, 3):
                ti = 0 if blk == 0 else 1
                yg0 = opool.tile([1, 512], F32, tag="yg0t")
                nc.scalar.copy(yg0[:], y_g[0][0:1, :])
                nc.sync.dma_start(yt_d[ti, 0:1], yg0[:])
                yg1 = opool.tile([1, 512], F32, tag="yg1t")
                nc.scalar.copy(yg1[:], y_g[1][0:1, :])
                nc.sync.dma_start(yt_d[ti, 1:2], yg1[:])
                nc.sync.dma_start(yt_d[ti, 2:3], y_sb[0:1, 512:1024])

        stage_front(0)
        for blk in range(1, N_BLK):
            stage_front(blk)
            stage_back(blk - 1)
        stage_back(N_BLK - 1)

    nc.compile()
    return nc


def kernel(x, Q, b, c):
    global _CACHED_NC
    x16 = np.ascontiguousarray(np.asarray(x, dtype=np.float32).astype(np.float16))
    Q64 = np.asarray(Q, dtype=np.float64)
    b64 = np.asarray(b, dtype=np.float64)
    c64 = float(np.asarray(c, dtype=np.float64).reshape(-1)[0])

    absQ = np.abs(Q64)
    # ill-conditioned columns: completion amplifies b^2/(4|Q|); keep device-side
    # values bounded and fix up exactly on host.
    with np.errstate(divide="ignore", invalid="ignore"):
        amp = np.where(absQ > 0, b64 * b64 / (4 * absQ), np.inf)
    bad = (amp > 2000.0) | (absQ == 0.0)

    sgn = np.where(np.asarray(Q) >= 0, 1.0, -1.0).astype(np.float32)
    s = np.sqrt(absQ).astype(np.float32)
    with np.errstate(divide="ignore", invalid="ignore"):
        t = (sgn.astype(np.float64) * b64 / (2 * s.astype(np.float64))).astype(np.float32)
    sgn[bad] = 0.0
    s[bad] = 0.0
    t[bad] = 0.0
    K = np.float64(c64 - np.sum(sgn.astype(np.float64) * t.astype(np.float64) ** 2))

    prm = np.zeros((128, 12), dtype=np.float32)
    prm[:, 0:4] = s.reshape(4, 128).T
    prm[:, 4:8] = t.reshape(4, 128).T
    prm[:, 8:12] = sgn.reshape(4, 128).T

    if _CACHED_NC is None:
        _CACHED_NC = _build_nc()
    nc = _CACHED_NC

    in_maps = [
        {"x": x16[i * N_PC : (i + 1) * N_PC], "prm": prm}
        for i in range(N_CORES)
    ]
    out = run_bass_kernel_spmd(nc, in_maps, core_ids=list(range(N_CORES)))
    y = np.concatenate([r["y"].reshape(-1) for r in out.results]).astype(np.float64)
    y += K

    if bad.any():
        idx = np.nonzero(bad)[0]
        xs = np.asarray(x, dtype=np.float32)[:, idx].astype(np.float64)
        corr = (xs * xs) @ Q64[idx] + xs @ b64[idx]
        y = y + corr

    return y.reshape(N_TOTAL, 1).astype(np.float32)
